# revision 21
# baseline (speedup 1.0000x reference)
"""DBHead (non-local attention + binarize/threshold conv branches) on 8 trn2 cores.

Sharding: 8 shards = 4 batch x 2 row-halves. Core (b, s) computes output rows
[128s, 128s+128) of the [3, 256, 256] map for batch b. All per-core variation
(which rows, halo padding, query-row masking) is pushed into host-prepared
input data so ONE SPMD program serves all 8 cores.

Perf notes (v2): all matmul operands fp16 except the softmax path (E, V,
rowsums in f32r -- the compiler rejects mixed 32/16-bit matmul inputs, so V
must stay f32r to multiply E). Inputs DMA straight into fp16 SBUF tiles (no
stage+cast). The attention inner loop is software-pipelined in key-chunk
pairs (score pair jj+1 issues before the xn group of pair jj) so the PE never
waits on the scalar-engine exp, keeping the HAM clock-gate warm (2.4 GHz).
exp runs as one ACT instruction per 2-bank PSUM pair. The threshold branch
result stays in SBUF (no DRAM round trip).
"""
import sys, os
sys.path.insert(0, "/opt/trn_rl_repo")
import numpy as np
from contextlib import ExitStack

import concourse.bass as bass
import concourse.tile as tile
from concourse import mybir, bacc
from concourse.bass_utils import run_bass_kernel_spmd

F32 = mybir.dt.float32
F32R = mybir.dt.float32r
F16 = mybir.dt.float16
BF16 = mybir.dt.bfloat16
AFT = mybir.ActivationFunctionType
ALU = mybir.AluOpType

EPS = 1e-5
NQ = 2176  # 34 rows x 64 cols of query positions (33 real + 1 zero halo row)
QBLOCKS = [(0, 448), (448, 448), (896, 448), (1344, 448), (1792, 384)]
NKC = 32  # key chunks of 128 over 4096 positions
NP = NKC // 2  # key-chunk pairs

# wpack column offsets (all fp16, rows = contraction dim on partitions)
OFF_WQ = 0        # 2 chunks x 64
OFF_WK = 128
OFF_WA = 256      # 2 chunks x 256
OFF_BZW = 768     # 18 x 64  (tap*2+chunk)
OFF_THW = 1920
OFF_DW1BZ = 3072  # 4 taps x 64 (rows 0:64)
OFF_DW1TH = 3328
OFF_DW2BZ = 3584  # 4 cols (rows 0:64)
OFF_DW2TH = 3588
OFF_BA = 3592     # row 0: asm-conv bias [256]
WCOLS = 3848

# bpack columns
BP_BQ, BP_BK = 0, 1
BP_BZ_S1, BP_BZ_B1, BP_BZ_S2, BP_BZ_B2 = 2, 3, 4, 5
BP_TH_S1, BP_TH_B1, BP_TH_S2, BP_TH_B2 = 6, 7, 8, 9
BP_BZ_DB2, BP_TH_DB2 = 10, 11
BCOLS = 16

_CACHE = {}
LAST_RESULTS = None


def _branch_ir(nc, tc, wr, bpk, hc, pcv, pct, ppt, pads, offw3, s1, b1,
               offdw1, s2, b2, offdw2, db2, outt, piece_cb=None):
    """Conv3x3+BN+ReLU -> ConvT(2,2)+BN+ReLU -> ConvT(2,2) -> sigmoid.
    pads: 2 channel-chunk tiles [128, 34, 66] fp16. outt: [4, 4, 2048] f32.
    Each block's dw chain is deferred behind the next block's conv matmuls
    (and interleaved dw1-ahead-of-dw2) so the PE never waits on ACT.
    piece_cb(t, c0, c1) is invoked right after outt[:, t, c0:c1] is written,
    letting the caller stream out/post-process pieces during the matmuls."""
    pts = {}

    def make_chain(blk, h1c):
        h2cs = {}

        def chain():
            for kind, t in ((0, 0), (0, 1), (0, 2), (1, 0),
                            (0, 3), (1, 1), (1, 2), (1, 3)):
                if kind == 0:
                    ct = pct.tile([64, 512], F32, tag="ct", name="ct")
                    o = offdw1 + t * 64
                    nc.tensor.matmul(ct[:], lhsT=wr[0:64, o:o + 64], rhs=h1c[:],
                                     start=True, stop=True)
                    h2c = hc.tile([64, 512], F16, tag="h2c", name="h2c")
                    nc.scalar.activation(h2c[:], ct[:], AFT.Relu,
                                         bias=bpk[0:64, b2:b2 + 1],
                                         scale=bpk[0:64, s2:s2 + 1])
                    h2cs[t] = h2c
                else:
                    if blk % 2 == 0:
                        pts[t] = ppt.tile([4, 2, 512], F32, tag="pt", name="pt")
                    pt = pts[t]
                    nc.tensor.matmul(pt[:, blk % 2, :],
                                     lhsT=wr[0:64, offdw2:offdw2 + 4],
                                     rhs=h2cs[t][:], start=True, stop=True)
                    if blk % 2 == 1:
                        c0, c1 = (blk - 1) * 512, (blk + 1) * 512
                        nc.scalar.activation(
                            outt[:, t, c0:c1], pt[:],
                            AFT.Sigmoid, bias=bpk[0:4, db2:db2 + 1])
                        if piece_cb is not None:
                            piece_cb(t, c0, c1)

        return chain

    pend = None
    for blk in range(4):
        cv = pcv.tile([64, 512], F32, tag="cv", name="cv")
        for t in range(9):
            ky, kx = t // 3, t % 3
            for c in range(2):
                o = offw3 + (t * 2 + c) * 64
                nc.tensor.matmul(
                    cv[:], lhsT=wr[:, o:o + 64],
                    rhs=pads[c][:, blk * 8 + ky:blk * 8 + ky + 8, kx:kx + 64],
                    start=(t == 0 and c == 0), stop=(t == 8 and c == 1))
        h1c = hc.tile([64, 512], F16, tag="h1c", name="h1c")
        nc.scalar.activation(h1c[:], cv[:], AFT.Relu,
                             bias=bpk[0:64, b1:b1 + 1],
                             scale=bpk[0:64, s1:s1 + 1])
        if pend is not None:
            pend()
        pend = make_chain(blk, h1c)
    pend()


def _build():
    nc = bacc.Bacc("TRN2", target_bir_lowering=False, debug=False, num_devices=8)
    xin_d = nc.dram_tensor("xin", [256, 64, 64], F16, kind="ExternalInput").ap()
    xq_d = nc.dram_tensor("xq", [256, 34, 64], F16, kind="ExternalInput").ap()
    xpad_d = nc.dram_tensor("xpad", [256, 34, 66], F16, kind="ExternalInput").ap()
    qm_d = nc.dram_tensor("qmask", [1, NQ], F32, kind="ExternalInput").ap()
    wp_d = nc.dram_tensor("wpack", [128, WCOLS], F16, kind="ExternalInput").ap()
    bp_d = nc.dram_tensor("bpack", [128, BCOLS], F32, kind="ExternalInput").ap()
    out_d = nc.dram_tensor("out", [3, 4, 4, 2048], F32, kind="ExternalOutput").ap()

    with tile.TileContext(nc) as tc, ExitStack() as ctx:
        cp = ctx.enter_context(tc.tile_pool(name="const", bufs=1))
        pp = ctx.enter_context(tc.tile_pool(name="pads", bufs=1))

        wr = cp.tile([128, WCOLS], F16)
        bpk = cp.tile([128, BCOLS], F32)
        qm = cp.tile([1, NQ], F32)
        xpr = [pp.tile([128, 34, 66], F16, tag=f"xp{c}", name=f"xpr{c}")
               for c in range(2)]
        xnp = [pp.tile([128, 34, 66], F16, tag=f"xn{c}", name=f"xnp{c}")
               for c in range(2)]

        # ---- input DMAs (no staging: fp16 straight into SBUF) ----
        # spread over 4 queues so the threshold branch can start early:
        # wpack+xpad (its inputs) land first, xq/xin stream in behind
        nc.sync.dma_start(wr[:], wp_d[:])
        nc.scalar.dma_start(bpk[:], bp_d[:])
        nc.scalar.dma_start(qm[:], qm_d[:])
        # xpad split into row-halves so the first conv blocks start sooner
        for c in range(2):
            sl = slice(c * 128, (c + 1) * 128)
            eng = nc.gpsimd if c == 0 else nc.scalar
            for r0, r1 in ((0, 18), (18, 34)):
                eng.dma_start(
                    xpr[c][:, r0:r1].rearrange("p r c2 -> p (r c2)"),
                    xpad_d[sl, r0:r1].rearrange("p r c2 -> p (r c2)"))
        # zero xn pads fully; interior rewritten by attention blocks
        for c in range(2):
            nc.vector.memset(xnp[c][:], 0.0)

        # ones in three dtypes: fp16 (K=1 broadcasts in fp16 matmuls), bf16
        # (Z-sum lhsT against bf16 E), f32r (fold/broadcast against f32r
        # operands; memset can't write f32r, so stage through f32)
        ones16 = cp.tile([1, 128], F16)
        nc.vector.memset(ones16[:], 1.0)
        onesb = cp.tile([128, 1], BF16)
        nc.vector.memset(onesb[:], 1.0)
        ones_f = cp.tile([128, 128], F32)
        nc.vector.memset(ones_f[:], 1.0)
        ones = cp.tile([128, 128], F32R)
        nc.vector.tensor_copy(ones[:], ones_f[:])

        Tt = cp.tile([4, 4, 2048], F32, tag="Tt", name="Tt")

        with tc.tile_pool(name="att", bufs=1) as ap_:
            e1r = ap_.tile([64, NQ], F16)
            e2r = ap_.tile([64, 4096], F16)
            V = ap_.tile([128, NKC, 256], BF16)

            with tc.tile_pool(name="xr", bufs=1) as xp:
                xr = [xp.tile([128, 64, 64], F16, tag=f"xr{c}", name=f"xr{c}")
                      for c in range(2)]
                xqr = [xp.tile([128, 34, 64], F16, tag=f"xq{c}", name=f"xqr{c}")
                       for c in range(2)]
                dmae = [nc.sync, nc.gpsimd, nc.scalar, nc.gpsimd]
                for c in range(2):
                    sl = slice(c * 128, (c + 1) * 128)
                    dmae[c].dma_start(xqr[c][:].rearrange("p r c2 -> p (r c2)"),
                                      xq_d[sl].rearrange("p r c2 -> p (r c2)"))
                    dmae[2 + c].dma_start(xr[c][:].rearrange("p r c2 -> p (r c2)"),
                                          xin_d[sl].rearrange("p r c2 -> p (r c2)"))

                # threshold branch: independent of attention; runs while
                # attention inputs stream in and warms up the PE clock
                def th_piece(t, c0, c1):
                    nc.sync.dma_start(out_d[1][:, t, c0:c1], Tt[:, t, c0:c1])

                with tc.tile_pool(name="hct", bufs=3) as hct, \
                     tc.tile_pool(name="pcv0", bufs=2, space="PSUM") as pcv0, \
                     tc.tile_pool(name="pct0", bufs=2, space="PSUM") as pct0, \
                     tc.tile_pool(name="ppt0", bufs=2, space="PSUM") as ppt0:
                    _branch_ir(nc, tc, wr, bpk, hct, pcv0, pct0, ppt0, xpr,
                               OFF_THW, BP_TH_S1, BP_TH_B1, OFF_DW1TH,
                               BP_TH_S2, BP_TH_B2, OFF_DW2TH, BP_TH_DB2, Tt,
                               piece_cb=th_piece)

                xr_f = [t[:].rearrange("p r c2 -> p (r c2)") for t in xr]
                xq_f = [t[:].rearrange("p r c2 -> p (r c2)") for t in xqr]

                # ---- phase 1: e1 (queries), e2 (keys), V (values) ----
                # eviction work is split across engines so ACT isn't the
                # limiter: V quads + e2 pairs on ACT (big instructions
                # amortize the 352-cycle ACT overhead), e1 PReLU on the
                # vector engine (add-bias then max(x, 0.25x)).
                with tc.tile_pool(name="pe", bufs=2, space="PSUM") as pe, \
                     tc.tile_pool(name="pe2", bufs=1, space="PSUM") as pe2, \
                     tc.tile_pool(name="pv", bufs=2, space="PSUM") as pv, \
                     tc.tile_pool(name="p1s", bufs=3) as p1s:
                    def emit_e1(q0, w):
                        p = pe.tile([64, 512], F32, tag="pe", name="pe_t")
                        for c in range(2):
                            o = OFF_WQ + c * 64
                            nc.tensor.matmul(p[:, :w], lhsT=wr[:, o:o + 64],
                                             rhs=xq_f[c][:, q0:q0 + w],
                                             start=(c == 0), stop=(c == 1))
                        t1 = p1s.tile([64, 512], F32, tag="t1", name="t1")
                        nc.vector.tensor_scalar(t1[:, :w], p[:, :w],
                                                bpk[0:64, BP_BQ:BP_BQ + 1],
                                                None, ALU.add)
                        nc.vector.scalar_tensor_tensor(
                            e1r[:, q0:q0 + w], t1[:, :w], 0.25, t1[:, :w],
                            ALU.mult, ALU.max)

                    def emit_e2(kk):
                        p = pe2.tile([64, 2, 512], F32, tag="pe2", name="pe2_t")
                        for h in range(2):
                            k0 = kk * 1024 + h * 512
                            for c in range(2):
                                o = OFF_WK + c * 64
                                nc.tensor.matmul(p[:, h, :],
                                                 lhsT=wr[:, o:o + 64],
                                                 rhs=xr_f[c][:, k0:k0 + 512],
                                                 start=(c == 0), stop=(c == 1))
                        nc.scalar.activation(
                            e2r[:, kk * 1024:(kk + 1) * 1024].rearrange(
                                "p (a b) -> p a b", a=2),
                            p[:], AFT.Prelu,
                            bias=bpk[0:64, BP_BK:BP_BK + 1], alpha=0.25)

                    def emit_v(qq):
                        p = pv.tile([128, 4, 256], F32, tag="pv", name="pv_t")
                        for u in range(4):
                            j = 4 * qq + u
                            for c in range(2):
                                o = OFF_WA + c * 256
                                nc.tensor.matmul(p[:, u, :],
                                                 lhsT=xr_f[c][:, j * 128:(j + 1) * 128],
                                                 rhs=wr[:, o:o + 256],
                                                 start=(c == 0), stop=False)
                            nc.tensor.matmul(p[:, u, :], lhsT=ones16[:],
                                             rhs=wr[0:1, OFF_BA:OFF_BA + 256],
                                             start=False, stop=True)
                        nc.scalar.activation(V[:, 4 * qq:4 * qq + 4, :], p[:],
                                             AFT.Prelu, alpha=0.25)

                    kinds = {"e1": [(q0, w) for q0, w in QBLOCKS],
                             "e2": list(range(4)), "v": list(range(8))}
                    while any(kinds.values()):
                        for k in ("e2", "v", "e1"):
                            if kinds[k]:
                                it = kinds[k].pop(0)
                                if k == "e1":
                                    emit_e1(*it)
                                elif k == "e2":
                                    emit_e2(it)
                                else:
                                    emit_v(it)

            # ---- phase 2: attention, software-pipelined in chunk pairs ----
            # PSUM budget (8 banks): sc tag 2x2 banks, xn tag 3x1, z tag 1x1.
            # The softmax denominator accumulates ON THE PE (ones-matmuls into
            # the z bank) -- the vector/gpsimd engines are too slow for the
            # 8.9M-element E-sum and contend for SBUF ports. xn accumulators
            # are raw-evicted to SBUF right at block end so their banks
            # recycle; the 1/Z normalization multiply happens SBUF-side,
            # deferred into the next block's pair loop.
            with tc.tile_pool(name="psc", bufs=2, space="PSUM") as psc, \
                 tc.tile_pool(name="pxn", bufs=3, space="PSUM") as pxn, \
                 tc.tile_pool(name="eb", bufs=3) as eb, \
                 tc.tile_pool(name="rc", bufs=2) as rc:

                NBLK = len(QBLOCKS)
                state = {}
                tails = {}

                def make_tail(bi):
                    q0, w = QBLOCKS[bi]
                    bs = state[bi]
                    rows, r0 = w // 64, q0 // 64
                    st = {}

                    def stage0():
                        rb = psc.tile([128, 512], F32, tag="sc", name="rb")
                        nc.tensor.matmul(rb[:, :w], lhsT=ones[0:1, 0:128],
                                         rhs=bs["rrm"][:, :w],
                                         start=True, stop=True)
                        rbs = rc.tile([128, 512], F32, tag="rbs", name="rbs")
                        nc.scalar.activation(rbs[:, :w], rb[:, :w], AFT.Copy)
                        st["rbs"] = rbs

                    def stage1():
                        rbs = st["rbs"]
                        engs = [nc.vector, nc.gpsimd]
                        for t in range(2):
                            engs[t].tensor_mul(
                                xnp[t][:, r0:r0 + rows, 1:65],
                                bs["xnraw"][t][:, :w].rearrange(
                                    "p (r c2) -> p r c2", c2=64),
                                rbs[:, :w].rearrange("p (r c2) -> p r c2",
                                                     c2=64))

                    return [stage0, stage1]

                def score_pair(bi, jj):
                    q0, w = QBLOCKS[bi]
                    sc = psc.tile([128, 2, 512], F32, tag="sc", name="sc")
                    for u in range(2):
                        j = 2 * jj + u
                        nc.tensor.matmul(sc[:, u, :w],
                                         lhsT=e2r[:, j * 128:(j + 1) * 128],
                                         rhs=e1r[:, q0:q0 + w],
                                         start=True, stop=True)
                    state[bi]["sc"][jj] = sc

                def xn_pair(bi, jj):
                    q0, w = QBLOCKS[bi]
                    bs = state[bi]
                    sc = bs["sc"].pop(jj)
                    xn_ps = bs["xn_ps"]
                    E = eb.tile([128, 2, 512], BF16, tag="E", name="E")
                    nc.scalar.activation(E[:, :, :w], sc[:, :, :w], AFT.Exp)
                    for u in range(2):
                        j = 2 * jj + u
                        for t in range(2):
                            nc.tensor.matmul(xn_ps[t][:, :w],
                                             lhsT=V[:, j, t * 128:(t + 1) * 128],
                                             rhs=E[:, u, :w],
                                             start=(j == 0), stop=(j == NKC - 1))
                        if u == 0:
                            # even chunks: Z-sum on the PE (group closed by
                            # the racc fold in raw_evict)
                            nc.tensor.matmul(bs["z"][:, :w], lhsT=onesb[:],
                                             rhs=E[:, u, :w],
                                             start=(j == 0), stop=False)
                        else:
                            # odd chunks: Z-partials on the vector engine
                            if jj == 0:
                                nc.vector.tensor_copy(bs["racc"][:, :w],
                                                      E[:, u, :w])
                            else:
                                nc.vector.tensor_add(bs["racc"][:, :w],
                                                     bs["racc"][:, :w],
                                                     E[:, u, :w])

                def raw_evict(bi):
                    q0, w = QBLOCKS[bi]
                    bs = state[bi]
                    # close the Z group (fold vector-engine partials), free
                    # the z bank fast (reciprocal), then the xn banks
                    nc.tensor.matmul(bs["z"][:, :w], lhsT=ones[:, 0:1],
                                     rhs=bs["racc"][:, :w],
                                     start=False, stop=True)
                    rrow = rc.tile([1, 512], F32, tag="rrow", name="rrow")
                    nc.vector.reciprocal(rrow[:, :w], bs["z"][:, :w])
                    rrm = rc.tile([1, 512], F32R, tag="rrm", name="rrm")
                    nc.gpsimd.tensor_mul(rrm[:, :w], rrow[:, :w],
                                         qm[:, q0:q0 + w])
                    bs["rrm"] = rrm
                    bs["xnraw"] = []
                    for t in range(2):
                        xnraw = rc.tile([128, 512], F32, tag="xnraw",
                                        name="xnraw", bufs=4)
                        if t == 0:
                            nc.scalar.activation(xnraw[:, :w],
                                                 bs["xn_ps"][t][:, :w],
                                                 AFT.Copy)
                        else:
                            nc.vector.tensor_copy(xnraw[:, :w],
                                                  bs["xn_ps"][t][:, :w])
                        bs["xnraw"].append(xnraw)

                def ensure_block(bi):
                    state[bi] = {
                        "xn_ps": [pxn.tile([128, 512], F32, tag="xn",
                                           name="xnps") for _ in range(2)],
                        "z": pxn.tile([1, 512], F32, tag="z", name="z",
                                      bufs=1),
                        "racc": rc.tile([128, 512], F32R, tag="racc",
                                        name="racc"),
                        "sc": {},
                    }

                total = NBLK * NP
                for k in range(total + 1):
                    if k < total:
                        bi, jj = divmod(k, NP)
                        if jj == 0:
                            ensure_block(bi)
                        score_pair(bi, jj)
                        if bi > 0 and jj in (4, 6):
                            tails[bi - 1][(jj - 4) // 2]()
                    if k > 0:
                        bi0, jj0 = divmod(k - 1, NP)
                        xn_pair(bi0, jj0)
                        if jj0 == NP - 1:
                            raw_evict(bi0)
                            tails[bi0] = make_tail(bi0)
                for stage in tails[NBLK - 1]:
                    stage()

        # ---- phase 3: binarize branch (att pool closed; SBUF freed) ----
        # binary map + output DMA are streamed per piece from inside the
        # branch so no serial tail remains after the last matmul
        with tc.tile_pool(name="br", bufs=1) as bp_, \
             tc.tile_pool(name="hc", bufs=3) as hc, \
             tc.tile_pool(name="pcv", bufs=2, space="PSUM") as pcv, \
             tc.tile_pool(name="pct", bufs=2, space="PSUM") as pct, \
             tc.tile_pool(name="ppt", bufs=2, space="PSUM") as ppt:
            P = bp_.tile([4, 4, 2048], F32, tag="P", name="P")
            B = bp_.tile([4, 4, 2048], F32, tag="B", name="B")

            def bz_piece(t, c0, c1):
                Dc = hc.tile([4, 1024], F32, tag="Dc", name="Dc")
                nc.vector.tensor_sub(Dc[:], P[:, t, c0:c1], Tt[:, t, c0:c1])
                nc.scalar.activation(B[:, t, c0:c1], Dc[:], AFT.Sigmoid,
                                     scale=50.0)
                nc.sync.dma_start(out_d[0][:, t, c0:c1], P[:, t, c0:c1])
                nc.gpsimd.dma_start(out_d[2][:, t, c0:c1], B[:, t, c0:c1])

            _branch_ir(nc, tc, wr, bpk, hc, pcv, pct, ppt, xnp, OFF_BZW,
                       BP_BZ_S1, BP_BZ_B1, OFF_DW1BZ, BP_BZ_S2, BP_BZ_B2,
                       OFF_DW2BZ, BP_BZ_DB2, P, piece_cb=bz_piece)

    nc.compile()
    return nc


def _prep(inputs):
    """Host-side parameter prep shared by all cores (numpy, tiny)."""
    g = {k: np.asarray(v, np.float32) for k, v in inputs.items()}
    wpack = np.zeros((128, WCOLS), np.float32)
    wqT = g["wm1"].reshape(64, 256).T
    wpack[:, OFF_WQ:OFF_WQ + 64] = wqT[0:128]
    wpack[:, OFF_WQ + 64:OFF_WQ + 128] = wqT[128:256]
    wkT = g["wm2"].reshape(64, 256).T
    wpack[:, OFF_WK:OFF_WK + 64] = wkT[0:128]
    wpack[:, OFF_WK + 64:OFF_WK + 128] = wkT[128:256]
    waT = g["wa"].reshape(256, 256).T
    wpack[:, OFF_WA:OFF_WA + 256] = waT[0:128]
    wpack[:, OFF_WA + 256:OFF_WA + 512] = waT[128:256]
    for name, off in (("bz_cw", OFF_BZW), ("th_cw", OFF_THW)):
        w3 = g[name].transpose(2, 3, 1, 0).reshape(9, 256, 64)
        for t in range(9):
            for c in range(2):
                wpack[:, off + (t * 2 + c) * 64:off + (t * 2 + c) * 64 + 64] = \
                    w3[t, c * 128:(c + 1) * 128]
    # conv_transpose flips the kernel: tap (di,dj) uses w[1-di, 1-dj]
    for name, off in (("bz_dw1", OFF_DW1BZ), ("th_dw1", OFF_DW1TH)):
        d1 = g[name].reshape(4, 64, 64)[::-1]
        for t in range(4):
            wpack[0:64, off + t * 64:off + (t + 1) * 64] = d1[t]
    for name, off in (("bz_dw2", OFF_DW2BZ), ("th_dw2", OFF_DW2TH)):
        wpack[0:64, off:off + 4] = \
            g[name].transpose(2, 0, 1, 3).reshape(64, 4)[:, ::-1]
    wpack[0, OFF_BA:OFF_BA + 256] = g["ba"]

    bpack = np.zeros((128, BCOLS), np.float32)
    bpack[0:64, BP_BQ] = g["bm1"]
    bpack[0:64, BP_BK] = g["bm2"]
    for pre, (cs1, cb1, cs2, cb2, cdb2) in (
            ("bz", (BP_BZ_S1, BP_BZ_B1, BP_BZ_S2, BP_BZ_B2, BP_BZ_DB2)),
            ("th", (BP_TH_S1, BP_TH_B1, BP_TH_S2, BP_TH_B2, BP_TH_DB2))):
        inv1 = g[f"{pre}_g1"] / np.sqrt(g[f"{pre}_v1"] + EPS)
        bpack[0:64, cs1] = inv1
        bpack[0:64, cb1] = g[f"{pre}_b1"] - g[f"{pre}_m1"] * inv1
        inv2 = g[f"{pre}_g2"] / np.sqrt(g[f"{pre}_v2"] + EPS)
        bpack[0:64, cs2] = inv2
        bpack[0:64, cb2] = g[f"{pre}_b2"] + (g[f"{pre}_db1"] - g[f"{pre}_m2"]) * inv2
        bpack[0:4, cdb2] = float(g[f"{pre}_db2"][0])
    return g, wpack.astype(np.float16), bpack


def kernel(**inputs):
    global LAST_RESULTS
    if "nc" not in _CACHE:
        _CACHE["nc"] = _build()
    nc = _CACHE["nc"]
    g, wpack, bpack = _prep(inputs)
    x = g["x"]  # [4, 256, 64, 64]

    in_maps = []
    for core in range(8):
        b, s = core % 4, core // 4
        xq = np.zeros((256, 34, 64), np.float32)
        xpad = np.zeros((256, 34, 66), np.float32)
        qmask = np.ones((1, NQ), np.float32)
        if s == 0:
            xq[:, 1:34] = x[b][:, 0:33]
            xpad[:, 1:34, 1:65] = x[b][:, 0:33]
            qmask[0, 0:64] = 0.0
        else:
            xq[:, 0:33] = x[b][:, 31:64]
            xpad[:, 0:33, 1:65] = x[b][:, 31:64]
            qmask[0, 33 * 64:] = 0.0
        in_maps.append({"xin": np.ascontiguousarray(x[b]).astype(np.float16),
                        "xq": xq.astype(np.float16), "xpad": xpad.astype(np.float16),
                        "qmask": qmask, "wpack": wpack, "bpack": bpack})

    br = run_bass_kernel_spmd(
        nc, in_maps, core_ids=list(range(8)),
        trace=os.environ.get("KERNEL_TRACE", "0") == "1")
    LAST_RESULTS = br

    out = np.zeros((4, 3, 256, 256), np.float32)
    for core in range(8):
        b, s = core % 4, core // 4
        raw = br.results[core]["out"].reshape(3, 2, 2, 2, 2, 32, 64)
        # [ch, ei, ej, a, b, r, c] -> rows (r,a,ei), cols (c,b,ej)
        half = raw.transpose(0, 5, 3, 1, 6, 4, 2).reshape(3, 128, 256)
        out[b, :, 128 * s:128 * (s + 1), :] = half
    return out
